# revision 9
# baseline (speedup 1.0000x reference)
"""DiT self-attention Trainium2 kernel, 8-way head-parallel (tensor parallel).

Strategy (per spec sharding_hint):
  - QKV projections column-sharded over heads: each of the 8 cores computes
    its 2 heads (256 channels) for all B*S tokens.  RMSNorm needs full-row
    sum-of-squares -> tiny AllReduce of per-token partials ([2,S] f32/batch).
  - RoPE applied locally (channels permuted host-side so that real/imag
    halves live in separate 16-partition blocks, making the rotation a
    stream_shuffle + 2 mul + 1 add on DVE).
  - Attention per (batch, local head): S^T = K^T Q tiles -> exp on ACT ->
    P^T; PV with a ones column appended to V gives the softmax denominator
    for free (129th output column).
  - Attention outputs are resharded token-wise with an AllToAll per batch,
    then the output projection runs on the local 2*256 tokens with the full
    wo (row sharding), so no further reduction is needed.

All matmuls run in bf16 (fp32 PSUM accumulation); norms/softmax math fp32.
"""

import math
import os
import sys

for _p in ("/opt/trn_rl_repo",):
    if _p not in sys.path and os.path.isdir(_p):
        sys.path.insert(0, _p)

import ml_dtypes
import numpy as np

import concourse.bacc as bacc
import concourse.bass as bass
import concourse.mybir as mybir
import concourse.tile as tile
from concourse.bass_utils import run_bass_kernel_spmd

BF16 = mybir.dt.bfloat16
F32 = mybir.dt.float32
AF = mybir.ActivationFunctionType
ALU = mybir.AluOpType
NPBF16 = ml_dtypes.bfloat16

N_CORES = 8
B, S, C = 2, 2048, 2048
N_HEADS, D, DH = 16, 128, 64
EPS = 1e-6

# Derived tiling constants (128-partition tiles everywhere).
HL = N_HEADS // N_CORES      # local heads per core
CL = HL * D                  # local channels
KT = C // 128                # contraction tiles
ST = S // 128                # token tiles per batch
CHUNK = S // N_CORES         # a2a chunk rows per batch
TL = B * CHUNK               # local output tokens per core
SCALE = 1.0 / math.sqrt(D)

SWAP16 = [(i + 16) % 32 for i in range(32)]  # stream_shuffle half-pair swap


def _head_perm():
    """Channel permutation for q/k: within each head's 128 channels, each
    32-partition quadrant holds [16 reals | 16 imags] of 16 adjacent
    complex pairs, so the RoPE partner lives 16 partitions away."""
    perm = np.empty(128, np.int64)
    for r in range(128):
        qd, lane = divmod(r, 32)
        pair = 16 * qd + (lane % 16)
        perm[r] = 2 * pair + (1 if lane >= 16 else 0)
    return perm  # perm[r] = original within-head channel at partition r


PERM128 = _head_perm()
PAIR_OF_ROW = (PERM128 // 2)          # complex pair index per partition row
ROW_IS_IMAG = (PERM128 % 2).astype(bool)


def build_program(has_bias_qk, has_bias_v, has_g, has_mask):
    nc = bacc.Bacc(
        "TRN2",
        target_bir_lowering=False,
        debug=False,
        enable_asserts=True,
        num_devices=N_CORES,
    )

    xT = nc.dram_tensor("xT", [C, B * S], BF16, kind="ExternalInput")
    wqT = nc.dram_tensor("wqT", [C, CL], BF16, kind="ExternalInput")
    wkT = nc.dram_tensor("wkT", [C, CL], BF16, kind="ExternalInput")
    wvT = nc.dram_tensor("wvT", [C, CL], BF16, kind="ExternalInput")
    woT = nc.dram_tensor("woT", [C, C], BF16, kind="ExternalInput")
    cosD = nc.dram_tensor("cosD", [128, S], F32, kind="ExternalInput")
    sinD = nc.dram_tensor("sinD", [128, S], F32, kind="ExternalInput")
    bqk = (
        nc.dram_tensor("bqk", [128, 2 * HL], F32, kind="ExternalInput")
        if has_bias_qk
        else None
    )
    bvb = (
        nc.dram_tensor("bvb", [128, CL], F32, kind="ExternalInput")
        if has_bias_v
        else None
    )
    gqk = (
        nc.dram_tensor("gqk", [128, 2 * HL], F32, kind="ExternalInput")
        if has_g
        else None
    )
    maskkT = (
        nc.dram_tensor("maskkT", [B, 128, ST], F32, kind="ExternalInput")
        if has_mask
        else None
    )
    out_loc = nc.dram_tensor("out_loc", [TL, C], F32, kind="ExternalOutput")

    groups = [list(range(N_CORES))]

    with tile.TileContext(nc) as tc:
        from contextlib import ExitStack

        with ExitStack() as top:
            const = top.enter_context(tc.tile_pool(name="const", bufs=1))
            dram = top.enter_context(tc.tile_pool(name="dram", bufs=1, space="DRAM"))
            qkbf_p = top.enter_context(tc.tile_pool(name="qkbf", bufs=2 * 2 * HL))
            vext_p = top.enter_context(tc.tile_pool(name="vext", bufs=B * ST))
            # pools used only through QKV+rope; closed before attention
            projstk = ExitStack()
            rs_p = projstk.enter_context(tc.tile_pool(name="rs", bufs=2))

            # --- constants / weights resident in SBUF ---
            projc = projstk.enter_context(tc.tile_pool(name="projc", bufs=1))
            wq_sb = projc.tile([128, KT * CL], BF16)
            wk_sb = projc.tile([128, KT * CL], BF16)
            wv_sb = projc.tile([128, KT * CL], BF16)
            for w_sb, w_dr in ((wq_sb, wqT), (wk_sb, wkT), (wv_sb, wvT)):
                nc.sync.dma_start(
                    out=w_sb[:].rearrange("p (kt c) -> p kt c", kt=KT),
                    in_=w_dr[:].rearrange("(kt p) c -> p kt c", p=128),
                )
            cos_sb = projc.tile([128, S], F32)
            sin_sb = projc.tile([128, S], F32)
            nc.sync.dma_start(out=cos_sb[:], in_=cosD[:])
            nc.sync.dma_start(out=sin_sb[:], in_=sinD[:])
            ones_col = const.tile([128, 1], BF16)
            nc.vector.memset(ones_col[:], 1.0)
            eps_col = const.tile([2, 1], F32)
            nc.vector.memset(eps_col[:], EPS)
            if has_bias_qk:
                bqk_sb = projc.tile([128, 2 * HL], F32)
                nc.sync.dma_start(out=bqk_sb[:], in_=bqk[:])
            if has_bias_v:
                bvb_sb = projc.tile([128, CL], F32)
                nc.sync.dma_start(out=bvb_sb[:], in_=bvb[:])
            if has_g:
                gqk_sb = projc.tile([128, 2 * HL], F32)
                nc.sync.dma_start(out=gqk_sb[:], in_=gqk[:])
            if has_mask:
                maskk_sb = const.tile([128, B * ST], F32)
                nc.sync.dma_start(
                    out=maskk_sb[:].rearrange("p (b t) -> p b t", b=B),
                    in_=maskkT[:].rearrange("b p t -> p b t"),
                )

            # --- internal DRAM ---
            ar_in = [dram.tile([2, S], F32, name=f"ar_in{b}") for b in range(B)]
            ar_out = [dram.tile([2, S], F32, name=f"ar_out{b}") for b in range(B)]
            rs_dr = [dram.tile([2, S], F32, name=f"rs_dr{b}") for b in range(B)]
            a2a_in = [dram.tile([N_CORES, CHUNK, CL], BF16, name=f"a2a_in{b}") for b in range(B)]
            a2a_out = [dram.tile([N_CORES, CHUNK, CL], BF16, name=f"a2a_out{b}") for b in range(B)]

            qbf = [[None] * HL for _ in range(B)]
            kbf = [[None] * HL for _ in range(B)]
            vext = [[None] * ST for _ in range(B)]

            # =================== QKV + norm + rope, per batch ===================
            for b in range(B):
                with ExitStack() as bstk:
                    raw_p = bstk.enter_context(
                        tc.tile_pool(name=f"raw{b}", bufs=2 * HL)
                    )
                    q2_p = bstk.enter_context(tc.tile_pool(name=f"q2{b}", bufs=2))
                    ss_sb_p = bstk.enter_context(tc.tile_pool(name=f"ss{b}", bufs=1))
                    raw = {("q", ct): raw_p.tile([128, S], F32, name=f"rawq{b}_{ct}", tag="raw") for ct in range(HL)}
                    raw.update(
                        {("k", ct): raw_p.tile([128, S], F32, name=f"rawk{b}_{ct}", tag="raw") for ct in range(HL)}
                    )

                    for th in range(2):  # token halves of this batch
                        HS = S // 2
                        with tc.tile_pool(name=f"xk{b}{th}", bufs=1) as xk_p:
                            xk = xk_p.tile([128, KT * HS], BF16)
                            for kt in range(KT):
                                nc.sync.dma_start(
                                    out=xk[:, kt * HS : (kt + 1) * HS],
                                    in_=xT[
                                        kt * 128 : (kt + 1) * 128,
                                        b * S + th * HS : b * S + (th + 1) * HS,
                                    ],
                                )
                            # ---- Q / K projections (channel-major output) ----
                            with tc.tile_pool(
                                name=f"qkps{b}{th}", bufs=4, space="PSUM"
                            ) as qkps:
                                for tname, w_sb in (("q", wq_sb), ("k", wk_sb)):
                                    for ct in range(HL):
                                        ps = qkps.tile([128, HS], F32)
                                        for kt in range(KT):
                                            lhsT = w_sb[
                                                :,
                                                kt * CL + ct * 128 : kt * CL
                                                + (ct + 1) * 128,
                                            ]
                                            for sl in range(HS // 512):
                                                nc.tensor.matmul(
                                                    ps[:, sl * 512 : (sl + 1) * 512],
                                                    lhsT,
                                                    xk[
                                                        :,
                                                        kt * HS
                                                        + sl * 512 : kt * HS
                                                        + (sl + 1) * 512,
                                                    ],
                                                    start=(kt == 0),
                                                    stop=(kt == KT - 1),
                                                )
                                        dst = raw[(tname, ct)][
                                            :, th * HS : (th + 1) * HS
                                        ]
                                        col = ct + (0 if tname == "q" else HL)
                                        if has_bias_qk:
                                            nc.scalar.activation(
                                                dst,
                                                ps[:],
                                                AF.Copy,
                                                bias=bqk_sb[:, col : col + 1],
                                            )
                                        else:
                                            nc.scalar.activation(dst, ps[:], AF.Copy)
                            # ---- V projection (token-major) + sumsq ----
                            with tc.tile_pool(
                                name=f"vps{b}{th}", bufs=4, space="PSUM"
                            ) as vps, tc.tile_pool(
                                name=f"ssps{b}{th}", bufs=1, space="PSUM"
                            ) as ssps:
                                for tt in range(th * (ST // 2), (th + 1) * (ST // 2)):
                                    psv = vps.tile([128, CL], F32)
                                    toff = tt * 128 - th * HS
                                    for kt in range(KT):
                                        nc.tensor.matmul(
                                            psv[:],
                                            xk[
                                                :,
                                                kt * HS + toff : kt * HS + toff + 128,
                                            ],
                                            wv_sb[:, kt * CL : (kt + 1) * CL],
                                            start=(kt == 0),
                                            stop=(kt == KT - 1),
                                        )
                                    vx = vext_p.tile([128, HL * 129], BF16)
                                    vext[b][tt] = vx
                                    nc.vector.memset(vx[:], 1.0)
                                    for hl in range(HL):
                                        dst = vx[:, hl * 129 : hl * 129 + 128]
                                        src = psv[:, hl * 128 : (hl + 1) * 128]
                                        if has_bias_v:
                                            nc.vector.scalar_tensor_tensor(
                                                dst,
                                                src,
                                                1.0,
                                                bvb_sb[:, hl * 128 : (hl + 1) * 128],
                                                ALU.mult,
                                                ALU.add,
                                            )
                                        else:
                                            nc.vector.tensor_copy(dst, src)
                                    # per-token sum of squares partials
                                for tname in ("q", "k"):
                                    q2s = []
                                    for ct in range(HL):
                                        q2 = q2_p.tile([128, HS], BF16)
                                        nc.scalar.activation(
                                            q2[:],
                                            raw[(tname, ct)][:, th * HS : (th + 1) * HS],
                                            AF.Square,
                                        )
                                        q2s.append(q2)
                                    pss = ssps.tile([1, HS], F32)
                                    for ct in range(HL):
                                        for sl in range(HS // 512):
                                            nc.tensor.matmul(
                                                pss[:, sl * 512 : (sl + 1) * 512],
                                                ones_col[:],
                                                q2s[ct][:, sl * 512 : (sl + 1) * 512],
                                                start=(ct == 0),
                                                stop=(ct == HL - 1),
                                            )
                                    row = 0 if tname == "q" else 1
                                    sss = ss_sb_p.tile([1, HS], F32, bufs=2)
                                    nc.scalar.activation(sss[:], pss[:], AF.Copy)
                                    nc.sync.dma_start(
                                        out=ar_in[b][row, th * HS : (th + 1) * HS],
                                        in_=sss[:],
                                    )

                    # ---- AllReduce of sumsq partials, rsqrt, broadcast ----
                    nc.gpsimd.collective_compute(
                        "AllReduce",
                        ALU.add,
                        replica_groups=groups,
                        ins=[ar_in[b][:].opt()],
                        outs=[ar_out[b][:].opt()],
                    )
                    ss2 = ss_sb_p.tile([2, S], F32, tag="ssw", bufs=2)
                    nc.sync.dma_start(out=ss2[:], in_=ar_out[b][:])
                    # rsqrt(mean + eps) = exp(-0.5 * ln(sumsq/C + eps))
                    lns = ss_sb_p.tile([2, S], F32, tag="ssw", bufs=2)
                    nc.scalar.activation(
                        lns[:], ss2[:], AF.Ln, scale=1.0 / C, bias=eps_col[:]
                    )
                    rs2 = ss_sb_p.tile([2, S], F32, tag="ssw", bufs=2)
                    nc.scalar.activation(rs2[:], lns[:], AF.Exp, scale=-0.5)
                    nc.sync.dma_start(out=rs_dr[b][:], in_=rs2[:])

                    rs_b = {}
                    for row, tname in ((0, "q"), (1, "k")):
                        rt = rs_p.tile([128, S], F32)
                        nc.sync.dma_start(
                            out=rt[:],
                            in_=rs_dr[b][row : row + 1, :].to_broadcast([128, S]),
                        )
                        rs_b[tname] = rt

                    # ---- norm + rope -> bf16 q/k in [d, t] layout ----
                    with tc.tile_pool(name=f"rope{b}", bufs=1) as rope_p:
                        for tname, dstarr in (("q", qbf), ("k", kbf)):
                            for ct in range(HL):
                                qn = rope_p.tile([128, S], F32)
                                if has_g:
                                    col = ct + (0 if tname == "q" else HL)
                                    nc.vector.scalar_tensor_tensor(
                                        qn[:],
                                        raw[(tname, ct)][:],
                                        gqk_sb[:, col : col + 1],
                                        rs_b[tname][:],
                                        ALU.mult,
                                        ALU.mult,
                                    )
                                else:
                                    nc.vector.tensor_tensor(
                                        qn[:],
                                        raw[(tname, ct)][:],
                                        rs_b[tname][:],
                                        ALU.mult,
                                    )
                                ysw = rope_p.tile([128, S], F32)
                                nc.vector.stream_shuffle(ysw[:], qn[:], SWAP16)
                                t1 = rope_p.tile([128, S], F32)
                                nc.vector.tensor_tensor(
                                    t1[:], qn[:], cos_sb[:], ALU.mult
                                )
                                t2 = rope_p.tile([128, S], F32)
                                nc.vector.tensor_tensor(
                                    t2[:], ysw[:], sin_sb[:], ALU.mult
                                )
                                dst = qkbf_p.tile([128, S], BF16)
                                nc.vector.tensor_tensor(
                                    dst[:], t1[:], t2[:], ALU.add
                                )
                                dstarr[b][ct] = dst

            projstk.close()

            # =================== attention + output projection ===================
            with ExitStack() as astk:
                stps = astk.enter_context(
                    tc.tile_pool(name="stps", bufs=2, space="PSUM")
                )
                pvps = astk.enter_context(
                    tc.tile_pool(name="pvps", bufs=2, space="PSUM")
                )
                wops = astk.enter_context(
                    tc.tile_pool(name="wops", bufs=2, space="PSUM")
                )
                pt_p = astk.enter_context(tc.tile_pool(name="pt", bufs=20))
                rec_p = astk.enter_context(tc.tile_pool(name="rec", bufs=4))
                abf_p = astk.enter_context(tc.tile_pool(name="abf", bufs=4))
                at_p = astk.enter_context(tc.tile_pool(name="at", bufs=B * KT))
                wo_p = astk.enter_context(tc.tile_pool(name="wo", bufs=KT))
                osb_p = astk.enter_context(tc.tile_pool(name="osb", bufs=3))

                at_tiles = [[None] * KT for _ in range(B)]

                for b in range(B):
                    for hl in range(HL):
                        qh = qbf[b][hl]
                        kh = kbf[b][hl]
                        pts = []
                        for tk in range(ST):
                            pt = pt_p.tile([128, S], BF16)
                            pts.append(pt)
                            for h2 in range(S // 1024):
                                ps = stps.tile([128, 1024], F32)
                                for sl in range(2):
                                    nc.tensor.matmul(
                                        ps[:, sl * 512 : (sl + 1) * 512],
                                        kh[:, tk * 128 : (tk + 1) * 128],
                                        qh[
                                            :,
                                            (h2 * 2 + sl) * 512 : (h2 * 2 + sl + 1)
                                            * 512,
                                        ],
                                        start=True,
                                        stop=True,
                                    )
                                dst = pt[:, h2 * 1024 : (h2 + 1) * 1024]
                                nc.scalar.activation(dst, ps[:], AF.Exp, scale=SCALE)
                                if has_mask:
                                    nc.vector.tensor_scalar_mul(
                                        dst,
                                        dst,
                                        maskk_sb[:, b * ST + tk : b * ST + tk + 1],
                                    )
                        for tq in range(ST):
                            po = pvps.tile([128, 129], F32)
                            for tk in range(ST):
                                nc.tensor.matmul(
                                    po[:],
                                    pts[tk][:, tq * 128 : (tq + 1) * 128],
                                    vext[b][tk][:, hl * 129 : (hl + 1) * 129],
                                    start=(tk == 0),
                                    stop=(tk == ST - 1),
                                )
                            rec = rec_p.tile([128, 1], F32)
                            nc.vector.reciprocal(rec[:], po[:, 128:129])
                            abf = abf_p.tile([128, D], BF16)
                            nc.vector.tensor_scalar_mul(abf[:], po[:, 0:D], rec[:])
                            j, r0 = divmod(tq * 128, CHUNK)
                            nc.sync.dma_start(
                                out=a2a_in[b][j, r0 : r0 + 128, hl * D : (hl + 1) * D],
                                in_=abf[:],
                            )

                    # ---- reshard tokens and apply output projection ----
                    nc.gpsimd.collective_compute(
                        "AllToAll",
                        ALU.bypass,
                        replica_groups=groups,
                        ins=[a2a_in[b][:].opt()],
                        outs=[a2a_out[b][:].opt()],
                    )
                    for i in range(N_CORES):
                        for cs in range(HL):
                            ci = i * HL + cs
                            at = at_p.tile([128, CHUNK], BF16)
                            at_tiles[b][ci] = at
                            nc.sync.dma_start_transpose(
                                at[:], a2a_out[b][i, :, cs * 128 : (cs + 1) * 128]
                            )

                for b in range(B):
                    for half in range(2):
                        wo_sb = []
                        for ci in range(KT):
                            wt = wo_p.tile([128, C // 2], BF16)
                            nc.sync.dma_start(
                                out=wt[:],
                                in_=woT[
                                    ci * 128 : (ci + 1) * 128,
                                    half * (C // 2) : (half + 1) * (C // 2),
                                ],
                            )
                            wo_sb.append(wt)
                        for tt in range(CHUNK // 128):
                            pso = [wops.tile([128, 512], F32, name=f"pso{b}_{half}_{tt}_{q}", tag="pso") for q in range(C // 1024)]
                            for ci in range(KT):
                                lhsT = at_tiles[b][ci][:, tt * 128 : (tt + 1) * 128]
                                for q in range(C // 1024):
                                    nc.tensor.matmul(
                                        pso[q][:],
                                        lhsT,
                                        wo_sb[ci][:, q * 512 : (q + 1) * 512],
                                        start=(ci == 0),
                                        stop=(ci == KT - 1),
                                    )
                            for q in range(C // 1024):
                                osb = osb_p.tile([128, 512], F32)
                                nc.scalar.activation(osb[:], pso[q][:], AF.Copy)
                                nc.sync.dma_start(
                                    out=out_loc[
                                        b * CHUNK + tt * 128 : b * CHUNK
                                        + (tt + 1) * 128,
                                        half * (C // 2)
                                        + q * 512 : half * (C // 2)
                                        + (q + 1) * 512,
                                    ],
                                    in_=osb[:],
                                )

    nc.compile()
    return nc


def _rope_volume_np(freqs_cs, f_p, h_p, w_p):
    t_dim = DH - 2 * (DH // 3)
    s_dim = DH // 3
    a_cos = np.asarray(freqs_cs[..., 0], np.float32)
    a_sin = np.asarray(freqs_cs[..., 1], np.float32)

    def vol(a):
        at = np.broadcast_to(a[:f_p, None, None, :t_dim], (f_p, h_p, w_p, t_dim))
        ah = np.broadcast_to(
            a[None, :h_p, None, t_dim : t_dim + s_dim], (f_p, h_p, w_p, s_dim)
        )
        aw = np.broadcast_to(
            a[None, None, :w_p, t_dim + s_dim :], (f_p, h_p, w_p, s_dim)
        )
        return np.concatenate([at, ah, aw], axis=-1).reshape(f_p * h_p * w_p, DH)

    return vol(a_cos), vol(a_sin)


_PROGRAM_CACHE = {}
_RUNNER_CACHE = {}


def _make_runner(nc):
    """Build a cached jitted shard_map runner for the compiled Bass program.

    Mirrors bass2jax.run_bass_via_pjrt but keeps the jitted function and lets
    the caller reuse device-resident input buffers for steady-state timing.
    """
    import jax
    from jax.sharding import Mesh, PartitionSpec
    from jax.experimental.shard_map import shard_map
    import concourse.mybir as _mybir
    from concourse.bass2jax import (
        _bass_exec_p,
        install_neuronx_cc_hook,
        partition_id_tensor,
    )

    install_neuronx_cc_hook()
    partition_name = nc.partition_id_tensor.name if nc.partition_id_tensor else None

    in_names, out_names, out_avals = [], [], []
    zero_outs = []
    for alloc in nc.m.functions[0].allocations:
        if not isinstance(alloc, _mybir.MemoryLocationSet):
            continue
        name = alloc.memorylocations[0].name
        if alloc.kind == "ExternalInput":
            if name != partition_name:
                in_names.append(name)
        elif alloc.kind == "ExternalOutput":
            shape = tuple(alloc.tensor_shape)
            dtype = _mybir.dt.np(alloc.dtype)
            out_names.append(name)
            out_avals.append(jax.core.ShapedArray(shape, dtype))
            zero_outs.append(np.zeros(shape, dtype))
    n_params = len(in_names)
    all_in_names = list(in_names) + list(out_names)
    if partition_name is not None:
        all_in_names.append(partition_name)

    def _body(*args):
        operands = list(args)
        if partition_name is not None:
            operands.append(partition_id_tensor())
        outs = _bass_exec_p.bind(
            *operands,
            out_avals=tuple(out_avals),
            in_names=tuple(all_in_names),
            out_names=tuple(out_names),
            lowering_input_output_aliases=(),
            sim_require_finite=True,
            sim_require_nnan=True,
            nc=nc,
        )
        return tuple(outs)

    devices = jax.devices()[:N_CORES]
    mesh = Mesh(np.asarray(devices), ("core",))
    nin = n_params + len(out_names)
    sharded = jax.jit(
        shard_map(
            _body,
            mesh=mesh,
            in_specs=(PartitionSpec("core"),) * nin,
            out_specs=(PartitionSpec("core"),) * len(out_names),
            check_rep=False,
        ),
        keep_unused=True,
    )

    def run(in_maps, timing_iters=0):
        per_core = [[np.asarray(m[nm]) for nm in in_names] for m in in_maps]
        concat_in = [
            np.concatenate([per_core[c][i] for c in range(N_CORES)], axis=0)
            for i in range(n_params)
        ]
        concat_zeros = [
            np.zeros((N_CORES * z.shape[0], *z.shape[1:]), z.dtype)
            for z in zero_outs
        ]
        args = [jax.device_put(a) for a in (*concat_in, *concat_zeros)]
        out_arrs = sharded(*args)
        jax.block_until_ready(out_arrs)
        best_ns = None
        if timing_iters:
            import time as _time

            for _ in range(timing_iters):
                t0 = _time.perf_counter()
                o = sharded(*args)
                jax.block_until_ready(o)
                dt = (_time.perf_counter() - t0) * 1e9
                best_ns = dt if best_ns is None else min(best_ns, dt)
        results = [
            {
                name: np.asarray(out_arrs[i]).reshape(N_CORES, *out_avals[i].shape)[c]
                for i, name in enumerate(out_names)
            }
            for c in range(N_CORES)
        ]
        return results, best_ns

    return run


def kernel(
    x,
    freqs_cs,
    wq,
    bq,
    wk,
    bk,
    wv,
    bv,
    wo,
    bo,
    gq,
    gk,
    frame_mask,
    f_p,
    h_p,
    w_p,
):
    x = np.asarray(x, np.float32)
    freqs_cs = np.asarray(freqs_cs, np.float32)
    wq, wk, wv, wo = (np.asarray(w, np.float32) for w in (wq, wk, wv, wo))
    bq, bk, bv, bo = (np.asarray(v, np.float32) for v in (bq, bk, bv, bo))
    gq, gk = np.asarray(gq, np.float32), np.asarray(gk, np.float32)
    mask = np.asarray(frame_mask, bool)
    f_p, h_p, w_p = int(f_p), int(h_p), int(w_p)

    has_bias_qk = bool(np.any(bq) or np.any(bk))
    has_bias_v = bool(np.any(bv))
    has_g = not (np.all(gq == 1.0) and np.all(gk == 1.0))
    has_mask = not bool(mask.all())

    key = (has_bias_qk, has_bias_v, has_g, has_mask)
    if key not in _PROGRAM_CACHE:
        _PROGRAM_CACHE[key] = build_program(*key)
    nc = _PROGRAM_CACHE[key]

    # ---------------- host-side prep ----------------
    cos_vol, sin_vol = _rope_volume_np(freqs_cs, f_p, h_p, w_p)  # [S, DH]
    cosD = cos_vol[:, PAIR_OF_ROW].T.astype(np.float32).copy()  # [128, S]
    sinD = sin_vol[:, PAIR_OF_ROW].T.astype(np.float32).copy()
    sinD[~ROW_IS_IMAG, :] *= -1.0

    xT = np.ascontiguousarray(x.reshape(B * S, C).T).astype(NPBF16)
    woT = np.ascontiguousarray(wo.T).astype(NPBF16)

    in_maps = []
    for core in range(N_CORES):
        ch0 = core * CL
        qk_rows = np.concatenate(
            [ch0 + hl * D + PERM128 for hl in range(HL)]
        )  # permuted global channels for q/k
        v_rows = np.arange(ch0, ch0 + CL)
        m = {
            "xT": xT,
            "wqT": np.ascontiguousarray(wq[qk_rows, :].T).astype(NPBF16),
            "wkT": np.ascontiguousarray(wk[qk_rows, :].T).astype(NPBF16),
            "wvT": np.ascontiguousarray(wv[v_rows, :].T).astype(NPBF16),
            "woT": woT,
            "cosD": cosD,
            "sinD": sinD,
        }
        if has_bias_qk:
            bq_l = bq[qk_rows].reshape(HL, 128).T
            bk_l = bk[qk_rows].reshape(HL, 128).T
            m["bqk"] = np.ascontiguousarray(
                np.concatenate([bq_l, bk_l], axis=1)
            ).astype(np.float32)
        if has_bias_v:
            m["bvb"] = np.ascontiguousarray(
                np.broadcast_to(bv[v_rows][None, :], (128, CL))
            ).astype(np.float32)
        if has_g:
            gq_l = gq[qk_rows].reshape(HL, 128).T
            gk_l = gk[qk_rows].reshape(HL, 128).T
            m["gqk"] = np.ascontiguousarray(
                np.concatenate([gq_l, gk_l], axis=1)
            ).astype(np.float32)
        if has_mask:
            mk = mask.astype(np.float32).reshape(B, ST, 128).transpose(0, 2, 1)
            m["maskkT"] = np.ascontiguousarray(mk)
        in_maps.append(m)

    if key not in _RUNNER_CACHE:
        _RUNNER_CACHE[key] = _make_runner(nc)
    timing_iters = int(os.environ.get("ATTN_TIME_ITERS", "0"))
    results, best_ns = _RUNNER_CACHE[key](in_maps, timing_iters=timing_iters)
    kernel._last_time_ns = best_ns

    out = np.empty((B * S, C), np.float32)
    for core in range(N_CORES):
        o = results[core]["out_loc"]
        for b in range(B):
            out[b * S + core * CHUNK : b * S + (core + 1) * CHUNK, :] = o[
                b * CHUNK : (b + 1) * CHUNK, :
            ]
    if np.any(bo):
        out += bo[None, :]
    out = out.reshape(B, S, C)
    if has_mask:
        out = np.where(mask[:, :, None], out, 0.0)
    return out
